# revision 43
# baseline (speedup 1.0000x reference)
"""FHN dynamics (IMEX, 8 unrolled steps) on 8 Trainium2 NeuronCores.

Contract: kernel(**inputs) takes the FULL inputs (stimulus [4,4096,2048] f32,
scalars a/b/dt, n_steps) and returns the FULL outputs (response, v) exactly
like the jax reference. Sharding is fully data-parallel: the 4*4096=16384
(batch*seq) rows are split into 8 contiguous shards of 2048 rows; every op is
elementwise or a reduce over the last axis, so no cross-core communication.

Math (per element; state (v, m) with m = Id - dt*w, Id = dt*I):
    v_next = F(v) + m,            F(v) = (1+dt)*v - (dt/3)*v^3
    m_next = k1*m - k2*v_next + C,  C = (1-k1)*Id - k0
with k1 = 1/denom, k2 = dt*alpha/denom, k0 = dt*alpha*a/denom,
alpha = dt/TAU, denom = 1+alpha*b. For the default params the clips never
bind (max pre-clip |v_next| = 2.687 < 3 over a dense Id grid; |w| <= 2.0 < 3),
verified at build time by _fast_path_ok; otherwise we fall back to a clipped
baseline-style program.

Implementation: custom fused DVE ops (8-slice micro-op programs, registered
at import into dve_ops.OPS) collapse each v-update and each h-update
(h = (k1-k2)*m - k2*F(v), so m_next = h + C) into ONE fp32 DVE pass each:
  ABSMAX: out=|x|, accum_out=max(1e-6, rowmax|x|)   (reduce fused with abs)
  MKID:   Id = (x*rs)*(g + 1/9), rs = recip*0.9*dt  (gate combine, 1 pass)
  STEP2V/STEP2H: step-2 v/h as cubic polys of Id (1-src passes; v1=Id, m1
  folded into coefficients)
  FSTEP:  v' = ((1+dt) - (dt/3) v^2)*v + m
  HSTEP:  h  = ((dt/3) k2 v^2 - k2(1+dt))*v + (k1-k2)*m
Per tile [128x2048]: 21 DVE passes (vs ~34 for the op-per-pass version);
ACT does sigmoid gate, C affine, resp scaling; preamble of tile t+1 is
emitted before the step chain of tile t so the ACT sigmoid overlaps DVE.
"""

import functools
import math
import os
import sys

import numpy as np

for _p in ("/opt/trn_rl_repo", os.path.expanduser("~/.axon_site/_ro/trn_rl_repo")):
    if os.path.isdir(_p) and _p not in sys.path:
        sys.path.insert(0, _p)

import concourse.bass as bass
import concourse.bacc as bacc
import concourse.tile as tile
from concourse import mybir
from concourse.bass_utils import run_bass_kernel_spmd
import concourse.dve_ops as dve_ops
from concourse.dve_ops import DveOp
from concourse.dve_spec import Spec, Src0, Src1, C0, C1, C2, maxx, sq, lower
from concourse.dve_uop import DveOpSpec, AluOp

TAU = 12.5
THRESHOLD = 0.5

N_CORES = 8
FULL_SHAPE = (4, 4096, 2048)
COLS = 2048
ROWS_TOTAL = (FULL_SHAPE[0] * FULL_SHAPE[1] * FULL_SHAPE[2]) // COLS  # 16384
ROWS_PER_CORE = ROWS_TOTAL // N_CORES  # 2048
P = 128

F32 = mybir.dt.float32
Alu = mybir.AluOpType
Act = mybir.ActivationFunctionType


# --- custom DVE op registration (runtime, same-process: dve_table_for_ops
# and the bass2jax realize path both read dve_ops module state) -------------


def _register_op(name: str, spec: Spec, subdim: bool = False) -> DveOp:
    if name in dve_ops._SUB_OPCODE_FOR_NAME:
        for op in dve_ops.OPS:
            if op.name == name:
                return op
    op = DveOp(name, spec, subdim=subdim, uops_sha={})
    dve_ops.OPS.append(op)
    dve_ops._SUB_OPCODE_FOR_NAME[name] = (
        dve_ops._CUSTOM_DVE_ROW_BASE + len(dve_ops.OPS) - 1
    )
    dve_ops.CUSTOM_DVE_SPECS[name] = spec
    opcode = dve_ops.get_dve_sub_opcode(name)
    for ver in ("v3", "v4"):
        compiled = DveOpSpec(
            name=name,
            opcode=opcode,
            uops=lower(spec, ver=ver),
            rd1_en=dve_ops.has_src1(spec),
        )
        op.uops_sha[ver] = compiled.sha(ver)
    return op


def _ref_absmax(in0, in1, s0, s1, imm2):
    b = np.abs(in0.astype(np.float32))
    return b, np.maximum(
        np.float32(s0), b.reshape(b.shape[0], -1).max(axis=-1, keepdims=True)
    )


# out = |x|; accum_out = max(s0, rowmax |x|)
FHN_ABSMAX = _register_op(
    "FHN_ABSMAX",
    Spec(
        body=maxx(Src0, -Src0),
        accum=AluOp.MAX,
        accum_init=C0,
        reference=_ref_absmax,
    ),
)

# Id = (x*s0)*(g + imm2)  [+ s1]   (s0 = recip*0.9*dt per-partition)
FHN_MKID = _register_op(
    "FHN_MKID",
    Spec(
        body=(Src0 * C0) * (Src1 + C2) + C1,
        reference=lambda in0, in1, s0, s1, imm2: (
            (in0.astype(np.float32) * s0) * (in1 + imm2) + s1
        ),
    ),
)

# Id = (x*s0)*(g*imm2 + s1)   (s0 = recip per-partition; gate affine folded)
FHN_MKID2 = _register_op(
    "FHN_MKID2",
    Spec(
        body=(Src0 * C0) * (Src1 * C2 + C1),
        reference=lambda in0, in1, s0, s1, imm2: (
            (in0.astype(np.float32) * s0) * (in1 * imm2 + s1)
        ),
    ),
)

# v2 = (s0 - imm2*Id^2)*Id + s1
FHN_POLY3 = _register_op(
    "FHN_POLY3",
    Spec(
        body=(C0 - sq(Src0) * C2) * Src0 + C1,
        reference=lambda in0, in1, s0, s1, imm2: (
            (s0 - imm2 * in0.astype(np.float32) ** 2) * in0 + s1
        ),
    ),
)

# h2 = (imm2*Id^2 + s0)*Id + s1
FHN_POLY3B = _register_op(
    "FHN_POLY3B",
    Spec(
        body=(sq(Src0) * C2 + C0) * Src0 + C1,
        reference=lambda in0, in1, s0, s1, imm2: (
            (imm2 * in0.astype(np.float32) ** 2 + s0) * in0 + s1
        ),
    ),
)

# v' = (s0 - imm2*v^2)*v + m
FHN_FSTEP = _register_op(
    "FHN_FSTEP",
    Spec(
        body=(C0 - sq(Src0) * C2) * Src0 + Src1,
        reference=lambda in0, in1, s0, s1, imm2: (
            (s0 - imm2 * in0.astype(np.float32) ** 2) * in0 + in1
        ),
    ),
)

# h = (imm2*v^2 + s1)*v + m*s0
FHN_HSTEP = _register_op(
    "FHN_HSTEP",
    Spec(
        body=(sq(Src0) * C2 + C1) * Src0 + Src1 * C0,
        reference=lambda in0, in1, s0, s1, imm2: (
            (imm2 * in0.astype(np.float32) ** 2 + s1) * in0 + in1 * s0
        ),
    ),
)


def _q_of(in0, in1, s0):
    v = in0.astype(np.float32)
    return (s0 - v * v) * v + in1


# Fused DOUBLE-step (normalized coords; Q = (s0 - v^2)*v + m = v_{t+1}):
# v-double: out = (s1 - Q^2)*Q + imm2      (+ PE-side affine terms)
_Q = (C0 - sq(Src0)) * Src0 + Src1
FHN_D2V = _register_op(
    "FHN_D2V",
    Spec(
        body=(C1 - sq(_Q)) * _Q + C2,
        reference=lambda in0, in1, s0, s1, imm2: (
            lambda Q: (s1 - Q * Q) * Q + imm2
        )(_q_of(in0, in1, s0)),
    ),
)

# M-double: out = ((s1 - Q^2)*Q)*imm2     (+ PE-side affine terms)
FHN_D2M = _register_op(
    "FHN_D2M",
    Spec(
        body=((C1 - sq(_Q)) * _Q) * C2,
        reference=lambda in0, in1, s0, s1, imm2: (
            lambda Q: ((s1 - Q * Q) * Q) * imm2
        )(_q_of(in0, in1, s0)),
    ),
)


def _consts(a: float, b: float, dt: float):
    alpha = dt / TAU
    denom = 1.0 + alpha * b
    k1 = 1.0 / denom
    k2 = dt * alpha / denom
    k0 = dt * alpha * a / denom
    return alpha, denom, k1, k2, k0


@functools.lru_cache(maxsize=4)
def _fast_path_ok(a: float, b: float, dt: float, n_steps: int) -> bool:
    """No-clip validity over a dense Id grid: pre-clip |v_next| and |w| must
    stay clear of 3.0. G is ~300-Lipschitz so a 1e-6-spaced grid is sound."""
    if not (0.0 < dt <= 3.0) or n_steps > 64:
        return False
    alpha, denom, k1, k2, k0 = _consts(a, b, dt)
    Id = np.linspace(-dt, dt, 2_000_001)
    C = (1.0 - k1) * Id - k0
    v = Id.copy()  # v1 (pre-clip |v1| = |Id| <= dt <= 3)
    m = (1.0 - k2) * Id - k0  # m1
    if np.abs(v).max() > 2.95:
        return False
    for _ in range(n_steps - 1):
        vn = (1.0 + dt) * v - (dt / 3.0) * v**3 + m
        mn = k1 * m - k2 * vn + C
        if np.abs(vn).max() > 2.95 or np.abs(Id - mn).max() > 2.9 * dt:
            return False
        v, m = vn, mn
    return True


def build_program(a: float, b: float, dt: float, n_steps: int,
                  rows: int = ROWS_PER_CORE, cols: int = COLS,
                  repeat: int = 1):
    """Per-core Bass program, fast path (no clips; custom fused DVE ops).

    The per-step m-update m' = h + C is offloaded to the PE: a bf16
    zero-matmul (start=True) sets the PSUM has_written bits, the DVE HSTEP
    writes h into the bank, then an fp32 identity-matmul accumulates C on
    top (verified exact on HW). The step chain runs on column halves so the
    PE accumulate of one half hides behind the DVE passes of the other.
    """
    alpha, denom, k1, k2, k0 = _consts(a, b, dt)
    c3 = dt / 3.0

    nt = rows // P
    assert rows % P == 0
    assert n_steps >= 2
    assert cols % (2 * 512) == 0
    HC = cols // 2  # half width
    MMN = 512       # one PSUM bank of fp32 per matmul

    nc = bacc.Bacc(None)
    x_d = nc.declare_dram_parameter("x", [rows, cols], F32, isOutput=False)
    resp_d = nc.declare_dram_parameter("resp", [rows, cols], F32, isOutput=True)
    vout_d = nc.declare_dram_parameter("vout", [rows, cols], F32, isOutput=True)

    # [P,1] bias constants for ACT (activation() needs an AP for the bias)
    sig_bias_t = nc.alloc_sbuf_tensor("sig_bias_const", [P, 1], F32)
    nc.gpsimd.memset(sig_bias_t.ap(), -10.0 * THRESHOLD)
    # identity fp32 weights (iota col-row -> 0 on diagonal -> is_equal)
    iot = nc.alloc_sbuf_tensor("iota_diag", [P, P], mybir.dt.int32)
    nc.gpsimd.iota(iot.ap(), pattern=[[1, P]], base=0, channel_multiplier=-1)
    zw = nc.alloc_sbuf_tensor("zeros_w_bf16", [P, P], mybir.dt.bfloat16)
    nc.gpsimd.memset(zw.ap(), 0.0)
    zx = nc.alloc_sbuf_tensor("zeros_x_bf16", [P, MMN], mybir.dt.bfloat16)
    nc.gpsimd.memset(zx.ap(), 0.0)
    nc.all_engine_barrier()
    ident_t = nc.alloc_sbuf_tensor("ident_f32", [P, P], F32)
    nc.vector.tensor_scalar(ident_t.ap(), iot.ap(), 0, None, op0=Alu.is_equal)
    nc.all_engine_barrier()
    sig_bias = sig_bias_t.ap()
    ident = ident_t.ap()
    zw_ap, zx_ap = zw.ap(), zx.ap()

    use_pe = n_steps > 2

    with tile.TileContext(nc) as tc:
        with (
            tc.tile_pool(name="work", bufs=2) as wp,
            tc.tile_pool(name="ps", bufs=10) as ps,
            tc.tile_pool(name="mp", bufs=2, space="PSUM") as mpool,
        ):
            def wt(tag, bufs):
                return wp.tile([P, cols], F32, tag=tag, bufs=bufs, name=tag)

            def mtile(par, s):
                # 4 fixed [P, HC] PSUM regions: (parity, side) = 8 banks total
                return mpool.tile([P, HC], F32, tag=f"m{par}{s}", bufs=1,
                                  name=f"m{par}{s}")

            def bitset(mt):
                """Set has_written bits on mt's banks (content = zeros)."""
                for c in range(HC // MMN):
                    nc.tensor.matmul(mt[:, c * MMN:(c + 1) * MMN], zw_ap, zx_ap,
                                     start=True, stop=False,
                                     skip_group_check=True)

            def c_accum(mt, C, s):
                """mt += C-half via fp32 identity matmul (exact)."""
                for c in range(HC // MMN):
                    sl = slice(s * HC + c * MMN, s * HC + (c + 1) * MMN)
                    nc.tensor.matmul(mt[:, c * MMN:(c + 1) * MMN], ident,
                                     C[:, sl], start=False, stop=True,
                                     skip_group_check=True)

            state = {}

            def emit_pre(it):
                """DMA in + absmax/scale + sigmoid gate for tile `it`."""
                r0 = it * P
                x = wt("xio", 3)
                nc.gpsimd.dma_start(out=x, in_=x_d[r0:r0 + P, :])
                ax = wt("ax", 2)
                scale0 = ps.tile([P, 1], F32)
                nc.vector._custom_dve(
                    FHN_ABSMAX, out=ax, accum_out=scale0, in0=x, s0=1e-6,
                )
                recip = ps.tile([P, 1], F32)
                nc.vector.reciprocal(recip, scale0)
                g = wt("gate", 2)
                nc.scalar.activation(g, ax, Act.Sigmoid,
                                     bias=sig_bias, scale=10.0)
                state[it] = (x, g, recip, scale0)

            def emit_mkid(it):
                """Id + C for tile `it` (separate so sigmoid gets a head
                start of one full tile of DVE work)."""
                x, g, recip, scale0 = state[it]
                Id = wt("id", 2)
                nc.vector._custom_dve(
                    FHN_MKID2, out=Id, in0=x, in1=g, s0=recip, s1=0.1 * dt,
                    imm2=0.9 * dt,
                )
                C = None
                if use_pe:
                    C = wt("C", 2)
                    nc.scalar.activation(C, Id, Act.Copy, bias=-k0,
                                         scale=1.0 - k1)
                state[it] = (Id, C, scale0)

            def emit_steps(it):
                """Step chain + outputs for tile `it` (column halves)."""
                r0 = it * P
                Id, C, scale0 = state.pop(it)
                hs = (slice(0, HC), slice(HC, cols))

                if use_pe:
                    # set has_written once per region per tile: bits persist
                    # across DVE overwrites (only TensorE matmuls touch them),
                    # so later m-updates in this tile skip the bit-setter and
                    # the PE only runs the C-accumulates per step.
                    for par in (0, 1):
                        for s in (0, 1):
                            bitset(mtile(par, s))

                # step 2 from Id (v1 = Id, m1 = (1-k2)Id - k0 folded in).
                # h2 halves go first: their PE C-accums gate step 3, so give
                # the PE the duration of the independent v2 pass as slack.
                m = (None, None)
                if use_pe:
                    m = (mtile(0, 0), mtile(0, 1))
                    for s in (0, 1):
                        nc.vector._custom_dve(
                            FHN_POLY3B, out=m[s], in0=Id[:, hs[s]],
                            s0=(k1 - k2) * (1.0 - k2) - k2 * (1.0 + dt),
                            s1=-(k1 - k2) * k0, imm2=k2 * c3,
                        )
                        c_accum(m[s], C, s)
                v = wt("v", 4)
                nc.vector._custom_dve(
                    FHN_POLY3, out=v, in0=Id,
                    s0=2.0 + dt - k2, s1=-k0, imm2=c3,
                )

                for step in range(3, n_steps + 1):
                    vp, mp = v, m
                    if step < n_steps:
                        par = step % 2  # step2 used parity 0 -> alternate
                        m = (mtile(par, 0), mtile(par, 1))
                        for s in (0, 1):
                            nc.vector._custom_dve(
                                FHN_HSTEP, out=m[s], in0=vp[:, hs[s]],
                                in1=mp[s],
                                s0=k1 - k2, s1=-k2 * (1.0 + dt), imm2=k2 * c3,
                            )
                        v = wt("v", 4)
                        for s in (0, 1):
                            c_accum(m[s], C, s)
                            nc.vector._custom_dve(
                                FHN_FSTEP, out=v[:, hs[s]], in0=vp[:, hs[s]],
                                in1=mp[s],
                                s0=1.0 + dt, imm2=c3,
                            )
                    else:
                        v = wt("v", 4)
                        for s in (0, 1):
                            nc.vector._custom_dve(
                                FHN_FSTEP, out=v[:, hs[s]], in0=vp[:, hs[s]],
                                in1=mp[s],
                                s0=1.0 + dt, imm2=c3,
                            )

                resp = wt("resp", 2)
                nc.scalar.mul(resp, v, scale0)
                nc.gpsimd.dma_start(out=resp_d[r0:r0 + P, :], in_=resp)
                nc.gpsimd.dma_start(out=vout_d[r0:r0 + P, :], in_=v)

            import contextlib
            rep_ctx = tc.For_i(0, repeat, 1) if repeat > 1 else contextlib.nullcontext()
            with rep_ctx:
                emit_pre(0)
                for it in range(nt):
                    if it + 1 < nt:
                        emit_pre(it + 1)
                    emit_mkid(it)
                    emit_steps(it)

    nc.finalize()
    return nc


def build_program_d2(a: float, b: float, dt: float, n_steps: int,
                     rows: int = ROWS_PER_CORE, cols: int = COLS,
                     repeat: int = 1):
    """Double-step program: 9 DVE passes/tile (vs 15 in build_program).

    Normalized state vt = s*v, Mt = s*m with s = sqrt(dt/3) so the cubic is
    F~(x) = (1+dt)x - x^3 (no dt/3 constant -> fused double-step fits the
    8-stage DVE pipeline). Per level (2 steps), two fused DVE passes:
      Q   = (A - vt^2)*vt + Mt              (= vt_{t+1}, internal)
      Dv  = (A-k2 - Q^2)*Q + cv             -> v-bank (PSUM)
      MD  = ((A+k1-k2 - Q^2)*Q) * (-k2)     -> M-bank (PSUM)
    The remaining affine terms are PE float32r scaled-identity accumulates
    into the PSUM banks (1 cycle/row at free-size>=256):
      v-bank += k1*Mt + (1-k1)*It
      M-bank += k1*(k1-k2)*Mt + (1+k1-k2)*(1-k1)*It + cm*ones
    where It = s*Id. Tails for levels >= 2 read an ACT-made SBUF copy of the
    previous M-bank. Tiles are processed in 4 column quarters so the PSUM
    chain (slots A/B/C, bufs=2) fits in 6 of the 8 banks.
    """
    alpha, denom, k1, k2, k0 = _consts(a, b, dt)
    s = math.sqrt(dt / 3.0)
    A = 1.0 + dt
    A2v = 2.0 + dt - k2          # v2 cubic linear coeff
    c2v = -s * k0                # v2 cubic const
    kap = k1 * (1.0 - k2) + 1.0 - k1   # M2 = -k2*v2 + kap*It + cMp
    cMp = -s * k0 * (1.0 + k1)
    km = k1 * (k1 - k2)
    em = (1.0 + k1 - k2) * (1.0 - k1)
    cm = -(1.0 + k1 - k2) * s * k0
    assert abs(kap) > 1e-3
    # Idc := kap*It + cMp is the level-1 Src1 AND (as f32r) the PE moving
    # tensor for all It-coefficient accumulates; weights divide by kap and
    # the stray (coef/kap)*cMp constants are compensated in the D2V imm2
    # (v-banks) or the ones-matmul coefficient (M-banks).
    ev1 = k1 * kap + 1.0 - k1    # It coeff into v4-bank (incl. k1*M2 tail)
    ev2 = 1.0 - k1               # It coeff into v-banks, levels >= 2
    em1 = km * kap + em          # It coeff into M4-bank (incl. km*M2 tail)
    i1 = -s * k0 + k1 * cMp - (ev1 / kap) * cMp   # D2V imm2, level 1
    i2 = -s * k0 - (ev2 / kap) * cMp              # D2V imm2, levels >= 2
    cm1 = cm + km * cMp - (em1 / kap) * cMp       # ones coeff, M4-bank
    cm2 = cm - (em / kap) * cMp                   # ones coeff, levels >= 2

    nt = rows // P
    assert rows % P == 0
    assert n_steps >= 4 and n_steps % 2 == 0
    L = (n_steps - 2) // 2  # number of double-levels after (v2, M2)
    NQ = 2                  # column halves
    W = cols // NQ          # 1024
    assert W >= 256 and W % 2 == 0  # fp32r 1cyc/row needs >=256

    F32R = mybir.dt.float32r
    nc = bacc.Bacc(None)
    x_d = nc.declare_dram_parameter("x", [rows, cols], F32, isOutput=False)
    resp_d = nc.declare_dram_parameter("resp", [rows, cols], F32, isOutput=True)
    vout_d = nc.declare_dram_parameter("vout", [rows, cols], F32, isOutput=True)

    sig_bias_t = nc.alloc_sbuf_tensor("sig_bias_const", [P, 1], F32)
    nc.gpsimd.memset(sig_bias_t.ap(), -10.0 * THRESHOLD)
    idc_bias_t = nc.alloc_sbuf_tensor("idc_bias_const", [P, 1], F32)
    nc.gpsimd.memset(idc_bias_t.ap(), cMp)
    iot = nc.alloc_sbuf_tensor("iota_diag", [P, P], mybir.dt.int32)
    nc.gpsimd.iota(iot.ap(), pattern=[[1, P]], base=0, channel_multiplier=-1)
    zw = nc.alloc_sbuf_tensor("zeros_w_bf16", [P, P], mybir.dt.bfloat16)
    nc.gpsimd.memset(zw.ap(), 0.0)
    zx = nc.alloc_sbuf_tensor("zeros_x_bf16", [P, W], mybir.dt.bfloat16)
    nc.gpsimd.memset(zx.ap(), 0.0)
    ones_f = nc.alloc_sbuf_tensor("ones_f32", [P, W], F32)
    nc.gpsimd.memset(ones_f.ap(), 1.0)
    nc.all_engine_barrier()
    ident_t = nc.alloc_sbuf_tensor("ident_f32", [P, P], F32)
    nc.vector.tensor_scalar(ident_t.ap(), iot.ap(), 0, None, op0=Alu.is_equal)
    # ones in genuine fp32r form (ACT copy performs the fp32->fp32r rounding)
    ones_rt = nc.alloc_sbuf_tensor("ones_f32r", [P, W], F32R)
    nc.scalar.copy(ones_rt.ap(), ones_f.ap())
    nc.all_engine_barrier()

    # coefficient-scaled identity weight matrices, rounded to float32r
    coefs = {
        "k1": k1,                   # McR tail into v-banks (lev >= 2)
        "km": km,                   # McR tail into M-banks (lev >= 2)
        "vk": -k1 * k2,             # v2r tail into v4-bank (lev 1)
        "vkm": -km * k2,            # v2r tail into M4-bank (lev 1)
        "ev1": ev1 / kap,           # Idc into v4-bank
        "ev2": ev2 / kap,           # Idc into v-banks, lev >= 2
        "em1": em1 / kap,           # Idc into M4-bank
        "em2": em / kap,            # Idc into M-banks, lev >= 2
        "cm1": cm1,                 # ones into M4-bank
        "cm2": cm2,                 # ones into M-banks, lev >= 2
    }
    wmat = {}
    wtmp = {}
    for nm, cf in coefs.items():
        t = nc.alloc_sbuf_tensor(f"w_{nm}", [P, P], F32)
        nc.vector.tensor_scalar(t.ap(), ident_t.ap(), float(cf), None,
                                op0=Alu.mult)
        wtmp[nm] = t
    nc.all_engine_barrier()
    for nm in coefs:
        tr = nc.alloc_sbuf_tensor(f"w_{nm}_r", [P, P], F32R)
        nc.scalar.copy(tr.ap(), wtmp[nm].ap())
        wmat[nm] = tr.ap()
    nc.all_engine_barrier()
    sig_bias = sig_bias_t.ap()
    idc_bias = idc_bias_t.ap()
    zw_ap, zx_ap = zw.ap(), zx.ap()
    ones_r = ones_rt.ap()

    MMN = 512  # one PSUM bank of fp32 per matmul (no bank crossing)

    def mm(out, w, x_ap, start=False):
        n = out.shape[-1]
        for c0 in range(0, n, MMN):
            nc.tensor.matmul(out[:, c0:c0 + MMN], w, x_ap[:, c0:c0 + MMN],
                             start=start, stop=not start,
                             skip_group_check=True)

    with tile.TileContext(nc) as tc:
        with (
            tc.tile_pool(name="work", bufs=2) as wp,
            tc.tile_pool(name="ps", bufs=10) as ps,
            tc.tile_pool(name="qp", bufs=2) as qp,
            tc.tile_pool(name="mp", bufs=2, space="PSUM") as mpool,
        ):
            def wt(tag, bufs):
                return wp.tile([P, cols], F32, tag=tag, bufs=bufs, name=tag)

            state = {}
            chain = {}

            def emit_pre(it):
                x = wt("xio", 3)
                nc.gpsimd.dma_start(out=x, in_=x_d[it * P:(it + 1) * P, :])
                ax = wt("ax", 2)
                scale0 = ps.tile([P, 1], F32)
                nc.vector._custom_dve(
                    FHN_ABSMAX, out=ax, accum_out=scale0, in0=x, s0=1e-6,
                )
                recip = ps.tile([P, 1], F32)
                nc.vector.reciprocal(recip, scale0)
                so = ps.tile([P, 1], F32)
                nc.vector.tensor_scalar(so, scale0, 1.0 / s, None, op0=Alu.mult)
                g = wt("gate", 2)
                nc.scalar.activation(g, ax, Act.Sigmoid,
                                     bias=sig_bias, scale=10.0)
                state[it] = (x, g, recip, so)

            def emit_mid_a1(it):
                x, g, recip, so = state.pop(it)
                # Idb = kap * It  (kap folded into the gate-affine consts)
                Idb = wt("id", 2)
                nc.vector._custom_dve(
                    FHN_MKID2, out=Idb, in0=x, in1=g, s0=recip,
                    s1=0.1 * dt * s * kap, imm2=0.9 * dt * s * kap,
                )
                # Idc = Idb + cMp = M2 + k2*v2: level-1 Src1 (ACT); with the
                # level-interleaved schedule it is ready a full level early
                Idc = wt("idc", 2)
                nc.scalar.activation(Idc, Idb, Act.Identity, bias=idc_bias)
                Idr = wp.tile([P, cols], F32R, tag="idr", bufs=2, name="idr")
                nc.scalar.copy(Idr, Idc)
                state[it] = (Idb, Idc, Idr, so)

            def emit_mid_a2(it):
                Idb, Idc, Idr, so = state.pop(it)
                v2 = wt("v2", 2)
                nc.vector._custom_dve(
                    FHN_POLY3, out=v2, in0=Idb,
                    s0=A2v / kap, s1=c2v, imm2=1.0 / (kap * kap * kap),
                )
                v2r = wp.tile([P, cols], F32R, tag="v2r", bufs=2, name="v2r")
                nc.scalar.copy(v2r, v2)
                resp = wt("resp", 2)
                vout = wt("vout", 2)
                state[it] = (Idc, Idr, v2, v2r, so, resp, vout)

            def emit_stage(it, q, lev):
                """Level lev (1..L) of half q: 2 fused DVE passes + PE."""
                Idc, Idr, v2, v2r, so, resp, vout = state[it]
                cs = slice(q * W, (q + 1) * W)
                Idq = Idr[:, cs]
                last = lev == L
                if lev == 1:
                    # Src1 = Idc = M2 + k2*v2; the -k2*v2 is folded into s0
                    vin, Min = v2[:, cs], Idc[:, cs]
                    Msrc = v2r[:, cs]
                    s0v = A - k2
                    wt_tail, wt_id, wt_cm = "vk", "ev1", "cm1"
                    immv = i1
                    Aq = mpool.tile([P, W], F32, tag="A", bufs=2, name="A")
                    mm(Aq, zw_ap, zx_ap, start=True)  # bitset A
                    chain[(it, q)] = [Aq, None, None]
                else:
                    Aq, McF, McR = chain[(it, q)]
                    vin, Min = Aq, McF
                    Msrc = McR
                    s0v = A
                    wt_tail, wt_id, wt_cm = "k1", "ev2", "cm2"
                    immv = i2
                if not last:
                    mtag = "B" if (q + lev) % 2 == 0 else "Cc"
                    Mq = mpool.tile([P, W], F32, tag=mtag, bufs=1, name=mtag)
                    mm(Mq, zw_ap, zx_ap, start=True)  # bitset M-bank
                    nc.vector._custom_dve(
                        FHN_D2M, out=Mq, in0=vin, in1=Min,
                        s0=s0v, s1=A + k1 - k2, imm2=-k2,
                    )
                nc.vector._custom_dve(
                    FHN_D2V, out=Aq, in0=vin, in1=Min,
                    s0=s0v, s1=A - k2, imm2=immv,
                )
                # PE affine accumulates (float32r identity matmuls)
                mm(Aq, wmat[wt_tail], Msrc)
                mm(Aq, wmat[wt_id], Idq)
                if not last:
                    mm(Mq, wmat["vkm" if lev == 1 else "km"], Msrc)
                    mm(Mq, wmat["em1" if lev == 1 else "em2"], Idq)
                    mm(Mq, wmat[wt_cm], ones_r)
                    # true M -> SBUF: fp32 for the next D-passes' Src1 (only
                    # one PSUM read is allowed per DVE op), fp32r for PE tails
                    McF2 = qp.tile([P, W], F32, tag="mf", bufs=2, name="mf")
                    nc.scalar.copy(McF2, Mq)
                    McR2 = qp.tile([P, W], F32R, tag="mr", bufs=2, name="mr")
                    nc.scalar.copy(McR2, Mq)
                    chain[(it, q)] = [Aq, McF2, McR2]
                else:
                    nc.scalar.activation(resp[:, cs], Aq, Act.Copy, scale=so)
                    nc.scalar.mul(vout[:, cs], Aq, 1.0 / s)
                    del chain[(it, q)]

            def emit_outs(it):
                Idc, Idr, v2, v2r, so, resp, vout = state.pop(it)
                r0 = it * P
                nc.gpsimd.dma_start(out=resp_d[r0:r0 + P, :], in_=resp)
                nc.gpsimd.dma_start(out=vout_d[r0:r0 + P, :], in_=vout)

            def emit_lev(it, lev):
                for q in range(NQ):
                    emit_stage(it, q, lev)

            import contextlib
            rep_ctx = tc.For_i(0, repeat, 1) if repeat > 1 else contextlib.nullcontext()
            with rep_ctx:
                # software pipeline: the next tile's full-width DVE passes
                # (ABSMAX / MKID2+V2 / M2) are interleaved between this
                # tile's chain levels so each level's PE-accumulate +
                # ACT-copy chain gets 4-6us of DVE slack.
                emit_pre(0)
                emit_mid_a1(0)
                emit_mid_a2(0)
                for it in range(nt):
                    nxt = it + 1 < nt
                    if nxt:
                        emit_pre(it + 1)
                    for lev in range(1, L + 1):
                        emit_lev(it, lev)
                        if nxt and lev == 1:
                            emit_mid_a1(it + 1)
                        if nxt and lev == min(2, L):
                            emit_mid_a2(it + 1)
                    emit_outs(it)

    nc.finalize()
    return nc


def build_program_safe(a: float, b: float, dt: float, n_steps: int,
                       rows: int = ROWS_PER_CORE, cols: int = COLS,
                       repeat: int = 1):
    """Fallback (baseline) program: standard DVE ops, v-clip applied.
    Used when _fast_path_ok fails for unusual parameters."""
    alpha, denom, k1, k2, k0 = _consts(a, b, dt)
    c3 = dt / 3.0
    sqrt_c3 = math.sqrt(c3)

    nt = rows // P
    assert rows % P == 0

    nc = bacc.Bacc(None)
    x_d = nc.declare_dram_parameter("x", [rows, cols], F32, isOutput=False)
    resp_d = nc.declare_dram_parameter("resp", [rows, cols], F32, isOutput=True)
    vout_d = nc.declare_dram_parameter("vout", [rows, cols], F32, isOutput=True)

    sig_bias_t = nc.alloc_sbuf_tensor("sig_bias_const", [P, 1], F32)
    nc.gpsimd.memset(sig_bias_t.ap(), -10.0 * THRESHOLD)
    nc.all_engine_barrier()
    sig_bias = sig_bias_t.ap()

    with tile.TileContext(nc) as tc:
        with (
            tc.tile_pool(name="work", bufs=2) as wp,
            tc.tile_pool(name="ps", bufs=8) as ps,
        ):
            def wt(tag, bufs):
                return wp.tile([P, cols], F32, tag=tag, bufs=bufs, name=tag)

            import contextlib
            rep_ctx = tc.For_i(0, repeat, 1) if repeat > 1 else contextlib.nullcontext()
            with rep_ctx:
              for it in range(nt):
                r0 = it * P
                x = wt("xio", 3)
                nc.gpsimd.dma_start(out=x, in_=x_d[r0:r0 + P, :])

                rowmax = ps.tile([P, 1], F32)
                nc.vector.tensor_reduce(
                    out=rowmax, in_=x, axis=mybir.AxisListType.X,
                    op=Alu.max, apply_absolute_value=True,
                )
                scale = ps.tile([P, 1], F32)
                nc.vector.tensor_scalar_max(scale, rowmax, 1e-6)
                recip = ps.tile([P, 1], F32)
                nc.vector.reciprocal(recip, scale)

                ax = wt("gate", 3)
                nc.scalar.activation(ax, x, Act.Abs)
                g = wt("gate", 3)
                nc.scalar.activation(g, ax, Act.Sigmoid,
                                     bias=sig_bias, scale=10.0)
                G = wt("gate", 3)
                nc.scalar.activation(G, g, Act.Copy, bias=0.1 * dt, scale=0.9 * dt)

                nx = wt("xio", 3)
                nc.scalar.mul(nx, x, recip)
                mm = wt("id", 2)
                nc.vector.tensor_mul(mm, nx, G)

                Ct = wt("C", 2)
                nc.scalar.activation(Ct, mm, Act.Copy, bias=-k0, scale=1.0 - k1)

                assert dt <= 3.0
                v = mm
                m = mm
                if n_steps > 1:
                    m2 = wt("m", 3)
                    nc.scalar.activation(m2, mm, Act.Copy, bias=-k0, scale=1.0 - k2)
                    m = m2

                for step in range(2, n_steps + 1):
                    sq_t = wt("sq", 2)
                    nc.scalar.activation(sq_t, v, Act.Square, scale=sqrt_c3)
                    Pn = wt("P", 2)
                    nc.vector.scalar_tensor_tensor(Pn, sq_t, 1.0 + dt, v,
                                                   op0=Alu.subtract, op1=Alu.mult)
                    z = wt("zv", 4)
                    nc.vector.tensor_sub(z, m, Pn)
                    v = wt("zv", 4)
                    nc.vector.tensor_scalar(v, z, 3.0, -3.0,
                                            op0=Alu.min, op1=Alu.max)
                    if step < n_steps:
                        t = wt("t", 1)
                        nc.vector.scalar_tensor_tensor(t, v, -k2, Ct,
                                                       op0=Alu.mult, op1=Alu.add)
                        m2 = wt("m", 3)
                        nc.vector.scalar_tensor_tensor(m2, m, k1, t,
                                                       op0=Alu.mult, op1=Alu.add)
                        m = m2

                resp = wt("resp", 1)
                nc.scalar.mul(resp, v, scale)
                nc.gpsimd.dma_start(out=resp_d[r0:r0 + P, :], in_=resp)
                nc.gpsimd.dma_start(out=vout_d[r0:r0 + P, :], in_=v)

    nc.finalize()
    return nc


def build_timing_program(a: float, b: float, dt: float, n_steps: int,
                         repeat: int = 1):
    """Best builder for these params (used by kernel() and test.py timing)."""
    if (n_steps >= 4 and n_steps % 2 == 0 and n_steps <= 64
            and _fast_path_ok(a, b, dt, n_steps)):
        return build_program_d2(a, b, dt, n_steps, repeat=repeat)
    if n_steps >= 2 and _fast_path_ok(a, b, dt, n_steps):
        return build_program(a, b, dt, n_steps, repeat=repeat)
    return build_program_safe(a, b, dt, n_steps, repeat=repeat)


@functools.lru_cache(maxsize=4)
def _cached_program(a: float, b: float, dt: float, n_steps: int):
    return build_timing_program(a, b, dt, n_steps)


def kernel(stimulus, a, b, dt, n_steps):
    stim = np.ascontiguousarray(np.asarray(stimulus, dtype=np.float32))
    assert stim.shape == FULL_SHAPE, stim.shape
    a = float(np.asarray(a))
    b = float(np.asarray(b))
    dt = float(np.asarray(dt))
    n_steps = int(np.asarray(n_steps))

    if n_steps < 2:
        # trivial host path: v1 = Id (|Id| <= dt), v0 = 0
        scale = np.clip(np.max(np.abs(stim), axis=-1, keepdims=True), 1e-6, None)
        if n_steps <= 0:
            v = np.zeros_like(stim)
        else:
            gate = 1.0 / (1.0 + np.exp(-(np.abs(stim) - THRESHOLD) * 10.0))
            v = np.clip((stim / scale) * (0.1 + 0.9 * gate) * dt, -3.0, 3.0)
        return (v * scale).astype(np.float32), v.astype(np.float32)

    nc = _cached_program(a, b, dt, n_steps)

    shards = stim.reshape(N_CORES, ROWS_PER_CORE, COLS)
    in_maps = [{"x": shards[i]} for i in range(N_CORES)]
    res = run_bass_kernel_spmd(nc, in_maps, list(range(N_CORES))).results

    resp = np.concatenate([res[i]["resp"] for i in range(N_CORES)], axis=0)
    v = np.concatenate([res[i]["vout"] for i in range(N_CORES)], axis=0)
    return resp.reshape(FULL_SHAPE), v.reshape(FULL_SHAPE)

